# revision 20
# baseline (speedup 1.0000x reference)
"""CrossAttentionBlockLLaMA on 8 Trainium2 NeuronCores (Bass/Tile).

Sharding:
  - QKV + attention: tensor-parallel over heads (2 heads/core).
  - Output projection wo: row-sharded over heads; each core computes a
    partial h for ALL tokens, written window-major [8, D, TC]; a
    ReduceScatter sums partials and hands core r exactly h.T[:, tokens_r].
  - FFN + post-norm: token-parallel (TC tokens/core), full weights.

Perf structure (v3):
  - q/k/v and attention output o live entirely in SBUF; attention + wo
    keep the PE fed while the ReduceScatter of the other stream flies.
  - softmax reciprocal runs as a single fast DVE approx op (the exact
    InstReciprocal is ~9x slower and stalled the o_ps PSUM rotation,
    causing ~6us PE gaps + HAM re-throttles every unit).
  - wo partials DMA straight from PSUM to DRAM with an in-flight
    f32->f16 cast on the gpsimd (software-DGE) queue - no DVE staging.
  - FFN runs in fp8 (e4m3) DoubleRow matmuls: 2 k-tiles per PE pass,
    ~1.8x faster than the fp16 path. Host pre-scales w1/w3 by 16 and
    w2 by 4 to center weights in e4m3 range; the scales cancel in the
    sigmoid input scale and the zg quantization step, so PSUM results
    are true-scale. h is cast f16->f8 in-flight by the gpsimd DMA.

Layouts: host pre-transposes activations/weights so every matmul's
contraction dim is on SBUF partitions. attn_norm_w and 1/sqrt(HD) are
folded into wq/wk/wv host-side; per-token 1/rms factors are applied to
q/k/v on device. Attention matmuls fp16, FFN fp8, PSUM always fp32.

Self-contained: hardcodes shapes from the problem spec.
"""
import numpy as np

NCORES = 8
EPS = 1e-5
WS1 = 16.0   # host scale on w1/w3 (fp8 range centering)
WS2 = 4.0    # host scale on w2
WSQ = 16.0   # host scale on wq/wk/wv


class Cfg:
    def __init__(self, B=2, S=2048, D=2048, H=16, HD=128, FF=5632):
        self.B, self.S, self.D, self.H, self.HD, self.FF = B, S, D, H, HD, FF
        self.T = B * S                    # total tokens
        self.TC = self.T // NCORES        # tokens per core (phase 3)
        self.NQ = (H // NCORES) * HD      # per-core head dims
        self.DT = D // 128                # d-tiles
        self.FT = FF // 128               # ff-tiles
        self.NQT = self.NQ // 128         # per-core head-dim tiles
        self.TCH = min(512, self.T)       # phase-1 token chunk
        self.QCH = min(512, S)            # phase-2 query chunk
        self.TCW = min(512, self.TC)      # phase-3 / wo token chunk
        assert self.T % self.TCH == 0 and S % self.QCH == 0
        assert self.TC % self.TCW == 0 and S % 128 == 0
        assert HD == 128 and D % 128 == 0 and FF % 128 == 0


FULL = Cfg()


def build(cfg=FULL):
    import concourse.mybir as mybir
    import concourse.tile as tile
    from concourse import bacc

    F16 = mybir.dt.float16
    F32 = mybir.dt.float32
    F8 = mybir.dt.float8e4

    c = cfg
    nc = bacc.Bacc("TRN2", target_bir_lowering=False, debug=False,
                   num_devices=NCORES)

    ins = {}
    outs = {}
    for s in ("x", "y"):
        ins[f"{s}P"] = nc.dram_tensor(
            f"{s}P", [128, c.T // c.TCH, c.DT, c.TCH], F8,
            kind="ExternalInput").ap()
        for w in ("wq", "wk", "wv"):
            ins[f"{w}P_{s}"] = nc.dram_tensor(
                f"{w}P_{s}", [128, c.DT, c.NQ], F8,
                kind="ExternalInput").ap()
        ins[f"woP_{s}"] = nc.dram_tensor(
            f"woP_{s}", [128, c.NQT, c.D], F16, kind="ExternalInput").ap()
        # fp8 DoubleRow FFN weights: pair-sliceable along the contraction
        ins[f"w1P_{s}"] = nc.dram_tensor(
            f"w1P_{s}", [128, c.FT, c.DT, 128], F8, kind="ExternalInput").ap()
        ins[f"w3P_{s}"] = nc.dram_tensor(
            f"w3P_{s}", [128, c.FT, c.DT, 128], F8, kind="ExternalInput").ap()
        ins[f"w2P_{s}"] = nc.dram_tensor(
            f"w2P_{s}", [128, c.DT, c.FT, 128], F8, kind="ExternalInput").ap()
        ins[f"res_{s}"] = nc.dram_tensor(
            f"res_{s}", [128, c.DT, c.TC], F16, kind="ExternalInput").ap()
        ins[f"fnorm_{s}"] = nc.dram_tensor(
            f"fnorm_{s}", [128, c.DT], F32, kind="ExternalInput").ap()
        outs[s] = nc.dram_tensor(f"out_{s}", [c.D, c.TC], F32,
                                 kind="ExternalOutput").ap()

    with tile.TileContext(nc) as tc:
        _emit(tc, nc, c, ins, outs)
    nc.compile()
    return nc


def _emit(tc, nc, c, ins, outs):
    import concourse.mybir as mybir

    F16 = mybir.dt.float16
    F32 = mybir.dt.float32
    F8 = mybir.dt.float8e4
    AF = mybir.ActivationFunctionType
    DR = mybir.MatmulPerfMode.DoubleRow
    MUL = mybir.AluOpType.mult
    one_over_d = 1.0 / c.D

    with (
        tc.tile_pool(name="psum", bufs=1, space="PSUM") as ps,
        tc.tile_pool(name="const", bufs=1) as const,
        tc.tile_pool(name="dram", bufs=1, space="DRAM") as dram,
    ):
        ones_col = const.tile([128, 1], F16)
        nc.vector.memset(ones_col[:], 1.0)
        eps1 = const.tile([128, 1], F32)
        nc.vector.memset(eps1[:], EPS)
        # phase-1 variant: rsq absorbs the 1/WSQ weight descale:
        # sqrt(su*(WSQ^2/D) + eps*WSQ^2) = WSQ*sqrt(su/D + eps)
        epsq = const.tile([128, 1], F32)
        nc.vector.memset(epsq[:], EPS * WSQ * WSQ)

        sc = {}
        nkb = c.S // 128  # key tiles per batch
        for s in ("x", "y"):
            # v in partition-major per-head layout, split per batch so
            # phase-2 loads only depend on that batch's phase-1 chunks
            for b in range(c.B):
                sc[f"v_{s}{b}"] = dram.tile([128, c.NQT, nkb, 128], F16,
                                            name=f"v_{s}{b}")
            # wo partials, window-major, partition-major rows: core w's
            # shard is rows [w*128,(w+1)*128) = a contiguous 2 MB block
            sc[f"hp_{s}"] = dram.tile([NCORES * 128, c.DT * c.TC], F16,
                                      name=f"hp_{s}")
            sc[f"h_{s}"] = dram.tile([128, c.DT, c.TC], F16,
                                     name=f"h_{s}")

        from concourse import bass_isa

        def mm(shape, name):
            return ps.tile(shape, F32, tag="mm", bufs=6, name=name)

        def omm(shape, name):
            return ps.tile(shape, F32, tag="omm", bufs=2, name=name)

        def col_rsqrt(acc16, width, sb_pool, name, scale, bias=None,
                      bufs=1):
            """Column-wise 1/sqrt(scale*colsum(acc)+eps), result in ALL
            partitions. The partition reduction runs on the idle GpSimd
            engine instead of the PE."""
            su = sb_pool.tile([128, width], F32, tag="cs_sum",
                              bufs=max(bufs, 2), name=f"sum_{name}")
            nc.gpsimd.partition_all_reduce(su[:], acc16[:],
                                           channels=128,
                                           reduce_op=bass_isa.ReduceOp.add)
            rt = sb_pool.tile([128, width], F32, tag="cs_rms",
                              bufs=max(bufs, 2), name=f"rms_{name}")
            nc.scalar.activation(rt[:], su[:], AF.Sqrt,
                                 bias=(eps1 if bias is None else bias)[:],
                                 scale=scale)
            rq = sb_pool.tile([128, width], F32, tag="cs_rsq",
                              bufs=max(bufs, 2), name=f"rsq_{name}")
            nc.vector.reciprocal_approx_fast(rq[:], rt[:])
            return rq

        # small long-lived pool: cross-phase prefetch targets
        with tc.tile_pool(name="pfp", bufs=1) as pfp:
          pf = {}

          NPF = 3  # (w1,w3) fb-pairs prefetched before the RS-y HBM storm

          def prefetch_p3(s):
            """First-chunk loads for phase 3, issued from the gpsimd queue
            right after the ReduceScatter trigger so they land while the
            PE still works on phase-2 compute (and before the second RS
            saturates HBM)."""
            fnorm = pfp.tile([128, c.DT], F32, tag="fnorm", bufs=2,
                             name=f"fnorm_{s}")
            nc.gpsimd.dma_start(fnorm[:], ins[f"fnorm_{s}"])
            pf[s] = {"fnorm": fnorm}
            if s != "x":
                return
            for fb in range(NPF):
                w1 = pfp.tile([128, 2, c.DT, 128], F8, tag=f"w1c{fb}",
                              name=f"w1c{fb}")
                nc.gpsimd.dma_start(w1[:],
                                    ins[f"w1P_{s}"][:, 2 * fb:2 * fb + 2])
                w3 = pfp.tile([128, 2, c.DT, 128], F8, tag=f"w3c{fb}",
                              name=f"w3c{fb}")
                nc.gpsimd.dma_start(w3[:],
                                    ins[f"w3P_{s}"][:, 2 * fb:2 * fb + 2])
                pf[s][f"w1{fb}"] = w1
                pf[s][f"w3{fb}"] = w3

          with tc.tile_pool(name="qko", bufs=1) as qko:
            # persistent SBUF q/k per stream (8 MB total); v spills to DRAM
            QT, KT = {}, {}
            for s in ("x", "y"):
                QT[s] = qko.tile([128, c.NQT, c.T], F16, name=f"qT_{s}")
                KT[s] = qko.tile([128, c.NQT, c.T], F16, name=f"kT_{s}")

            # ======== PHASE 1: RMSNorm stats + QKV projections ========
            with (
                tc.tile_pool(name="p1w", bufs=1) as p1w,
                tc.tile_pool(name="p1a", bufs=2) as p1a,
                tc.tile_pool(name="p1s", bufs=2) as p1s,
            ):
                W = {}
                for s in ("x", "y"):
                    for w in ("wq", "wk", "wv"):
                        t = p1w.tile([128, c.DT, c.NQ], F8,
                                     name=f"{w}_{s}_sb")
                        # split loads so the first matmuls start early
                        for g in range(0, c.DT, 4):
                            nc.gpsimd.dma_start(
                                t[:, g:g + 4], ins[f"{w}P_{s}"][:, g:g + 4])
                        W[f"{w}{s}"] = t

                chunks_per_b = c.S // c.TCH

                def stats_chunk(ich):
                    """Load a chunk + compute its rms stats. Runs one
                    chunk AHEAD of the matmuls so the serial chain
                    (tree adds -> gpsimd reduce -> rsqrt) hides under
                    the previous chunk's PE work."""
                    st = {"act": {}, "rsq_free": {}, "rsq_part": {}}
                    for s in ("x", "y"):
                        at = p1a.tile([128, c.DT, c.TCH], F8,
                                      tag=f"act_{s}", name=f"act_{s}")
                        for g in range(0, c.DT, 4):
                            nc.sync.dma_start(at[:, g:g + 4],
                                              ins[f"{s}P"][:, ich, g:g + 4])
                        st["act"][s] = at

                        # mean-square: squares on the (idle) scalar
                        # engine, tree-accumulate on DVE (depth 4, not
                        # a 15-deep chain), reduce on gpsimd
                        sq = [None] * c.DT
                        for o in range(c.DT):
                            t = p1s.tile([128, c.TCH], F16, tag="sq",
                                         bufs=2 * c.DT + 2,
                                         name=f"sq_{s}{o}")
                            nc.scalar.activation(t[:], at[:, o],
                                                 AF.Square)
                            sq[o] = t
                        n = c.DT
                        while n > 1:
                            for i in range(n // 2):
                                nc.vector.tensor_add(sq[i][:], sq[i][:],
                                                     sq[n - 1 - i][:])
                            n = (n + 1) // 2
                        rq = col_rsqrt(sq[0], c.TCH, p1s, f"p1{s}",
                                       WSQ * WSQ * one_over_d, bias=epsq,
                                       bufs=4)
                        st["rsq_free"][s] = rq

                        nsub = c.TCH // 128
                        rfd = dram.tile([1, c.TCH], F32, tag="rsq_row",
                                        bufs=4, name=f"rfd_{s}")
                        nc.scalar.dma_start(rfd[:], rq[:1, :])
                        rsqT = p1s.tile([128, nsub], F32, tag="rsqT",
                                        bufs=4, name=f"rsqT_{s}")
                        nc.scalar.dma_start(
                            rsqT[:],
                            rfd[0, :].rearrange("(n p) -> p n", p=128))
                        st["rsq_part"][s] = rsqT
                    return st

                nxt = stats_chunk(0)
                for ich in range(c.T // c.TCH):
                    tsl = slice(ich * c.TCH, (ich + 1) * c.TCH)
                    cur = nxt
                    if ich + 1 < c.T // c.TCH:
                        nxt = stats_chunk(ich + 1)
                    act = cur["act"]
                    rsq_free = cur["rsq_free"]
                    rsq_part = cur["rsq_part"]

                    for s in ("x", "y"):
                        kv = "y" if s == "x" else "x"
                        bc_q = rsq_free[s]
                        bc_k = rsq_free[kv]

                        for (wname, src, bc, dst) in (
                            ("wq", s, bc_q, QT[s]),
                            ("wk", kv, bc_k, KT[s]),
                        ):
                            for jt in range(c.NQT):
                                pm = mm([128, c.TCH], f"{wname}{s}{jt}")
                                wt = W[f"{wname}{s}"]
                                for op in range(c.DT // 2):
                                    nc.tensor.matmul(
                                        pm[:],
                                        wt[:, 2 * op:2 * op + 2,
                                           jt * 128:(jt + 1) * 128],
                                        act[src][:, 2 * op:2 * op + 2, :],
                                        start=(op == 0),
                                        stop=(op == c.DT // 2 - 1),
                                        perf_mode=DR)
                                nc.vector.tensor_mul(dst[:, jt, tsl],
                                                     pm[:], bc[:])

                        nsub = c.TCH // 128
                        vw = p1s.tile([128, nsub, c.NQ], F16, tag="vw",
                                      bufs=1, name=f"vw_{s}")
                        for i in range(nsub):
                            pv = mm([128, c.NQ], f"v{s}{i}")
                            for op in range(c.DT // 2):
                                nc.tensor.matmul(
                                    pv[:],
                                    act[kv][:, 2 * op:2 * op + 2,
                                            i * 128:(i + 1) * 128],
                                    W[f"wv{s}"][:, 2 * op:2 * op + 2, :],
                                    start=(op == 0),
                                    stop=(op == c.DT // 2 - 1),
                                    perf_mode=DR)
                            nc.vector.tensor_scalar_mul(
                                vw[:, i, :], pv[:],
                                rsq_part[kv][:, i:i + 1])
                        vb = ich // chunks_per_b
                        icb = ich % chunks_per_b
                        for hh in range(c.NQT):
                            nc.sync.dma_start(
                                sc[f"v_{s}{vb}"][:, hh,
                                                 icb * nsub:(icb + 1) * nsub,
                                                 :],
                                vw[:, :, hh * 128:(hh + 1) * 128])

            # ======== PHASE 2: attention + wo partial + ReduceScatter ====
            with (
                tc.tile_pool(name="p2", bufs=2) as p2,
                tc.tile_pool(name="p2o", bufs=1) as p2o,
                tc.tile_pool(name="p2w", bufs=2) as p2w,
            ):
                nk = c.S // 128
                # hoist attention/wo input loads, ordered so the tiles the
                # first units need arrive first: VT(x,b0), WO(x), then rest
                VT, WO = {}, {}

                def load_vt(s, b):
                    for h in range(c.NQT):
                        vt = p2.tile([128, nk, 128], F16, tag="vt",
                                     bufs=8, name=f"vt_{s}{b}{h}")
                        nc.sync.dma_start(vt[:], sc[f"v_{s}{b}"][:, h])
                        VT[(s, b, h)] = vt

                def load_wo(s):
                    WO[s] = p2w.tile([128, c.NQT, c.D], F16, tag="wo",
                                     bufs=2, name=f"wo_{s}")
                    nc.sync.dma_start(WO[s][:], ins[f"woP_{s}"][:])

                load_vt("x", 0)
                load_wo("x")
                load_vt("x", 1)
                load_vt("y", 0)
                load_wo("y")
                load_vt("y", 1)

                for s in ("x", "y"):
                    # one shared o buffer; stream y reuses x's after wo-x
                    o_sb = p2o.tile([128, c.NQT, c.T], F16, tag="osb",
                                    bufs=1, name="o_sb")

                    # softmax normalize is software-pipelined one unit
                    # behind the matmuls; the unnormalized o is cast to
                    # SBUF at unit end (freeing the PSUM bank immediately
                    # so the next unit's accumulation never waits on the
                    # reciprocal chain) and normalized in place later
                    pend = None

                    def flush_recip(p):
                        rcp = p2.tile([128, c.QCH], F32, tag="cs_rsq",
                                      name="ercp")
                        nc.vector.reciprocal_approx_fast(rcp[:], p["su"][:])
                        p["rcp"] = rcp

                    def flush_mul(p):
                        nc.vector.tensor_mul(o_sb[:, p["h"], p["qsl"]],
                                             o_sb[:, p["h"], p["qsl"]],
                                             p["rcp"][:])

                    def wo_window(w):
                        # attention is exp(scalar)-throughput-bound; the wo
                        # matmuls interleave as pure-PE filler once both
                        # heads of window w are normalized. The window's
                        # partials batch into ONE contiguous 2 MB store;
                        # PSUM->SBUF casts split across DVE and Scalar so
                        # neither engine saturates.
                        t0 = w * c.TC
                        hpw = p2w.tile([128, c.DT, c.TCW], F16,
                                       tag="hpw", bufs=1, name="hpw")
                        for dt in range(c.DT):
                            hp = mm([128, c.TCW], "hp")
                            for o in range(c.NQT):
                                nc.tensor.matmul(
                                    hp[:],
                                    WO[s][:, o, dt * 128:(dt + 1) * 128],
                                    o_sb[:, o, t0:t0 + c.TCW],
                                    start=(o == 0),
                                    stop=(o == c.NQT - 1))
                            if dt % 2 == 0:
                                nc.vector.tensor_copy(hpw[:, dt, :], hp[:])
                            else:
                                nc.scalar.activation(hpw[:, dt, :], hp[:],
                                                     AF.Copy)
                        nc.gpsimd.dma_start(
                            sc[f"hp_{s}"][w * 128:(w + 1) * 128, :],
                            hpw[:])

                    units = [(b, q0, h)
                             for b in range(c.B)
                             for q0 in range(0, c.S, c.QCH)
                             for h in range(c.NQT)]
                    for j, (b, q0, h) in enumerate(units):
                        vt = VT[(s, b, h)]
                        qsl = slice(b * c.S + q0, b * c.S + q0 + c.QCH)
                        o_ps = omm([128, c.QCH], "o_ps")
                        e_acc = p2.tile([128, c.QCH], F16, tag="eacc",
                                        bufs=2, name="eacc")
                        for ik in range(nk):
                            s_ps = mm([128, c.QCH], "s_ps")
                            nc.tensor.matmul(
                                s_ps[:],
                                KT[s][:, h, b * c.S + ik * 128:
                                      b * c.S + (ik + 1) * 128],
                                QT[s][:, h, qsl],
                                start=True, stop=True)
                            e16 = p2.tile([128, c.QCH], F16, tag="e16",
                                          bufs=6, name="e16")
                            nc.scalar.activation(e16[:], s_ps[:], AF.Exp)
                            if ik == 0:
                                nc.vector.tensor_copy(e_acc[:], e16[:])
                            else:
                                nc.vector.tensor_add(e_acc[:], e_acc[:],
                                                     e16[:])
                            if ik == 8 and pend is not None:
                                flush_recip(pend)
                            nc.tensor.matmul(o_ps[:], vt[:, ik], e16[:],
                                             start=(ik == 0),
                                             stop=(ik == nk - 1))
                        su = p2.tile([128, c.QCH], F32, tag="cs_sum",
                                     name="esum")
                        nc.gpsimd.partition_all_reduce(
                            su[:], e_acc[:], channels=128,
                            reduce_op=bass_isa.ReduceOp.add)
                        # free the o PSUM bank now; normalize in place
                        # once the reciprocal lands (one unit later)
                        nc.vector.tensor_copy(o_sb[:, h, qsl], o_ps[:])
                        if pend is not None:
                            flush_mul(pend)
                        pend = {"su": su, "h": h, "qsl": qsl}
                        if j >= 3 and j % 2 == 1:
                            wo_window((j - 3) // 2)
                    flush_recip(pend)
                    flush_mul(pend)
                    pend = None
                    wo_window(NCORES - 1)

                    nc.gpsimd.collective_compute(
                        "ReduceScatter", mybir.AluOpType.add,
                        replica_groups=[list(range(NCORES))],
                        ins=[sc[f"hp_{s}"][:].opt()],
                        outs=[sc[f"h_{s}"][:].opt()],
                    )
                    prefetch_p3(s)

          # ======== PHASE 3: SwiGLU FFN + residual + post-norm ========
          # fp8 DoubleRow: one PE pass per k-tile pair. Host scales
          # w1/w3 by WS1 and w2 by WS2; sigmoid input rescales by 1/WS1
          # and the zg quantization by 1/(WS1*WS2), so PSUM results are
          # true-scale everywhere.
          with (
            tc.tile_pool(name="p3", bufs=1) as p3,
            tc.tile_pool(name="p3w", bufs=2) as p3w,
            tc.tile_pool(name="p3s", bufs=2) as p3s,
          ):
            for s in ("x", "y"):
                fnorm = pf[s]["fnorm"]
                for icw in range(c.TC // c.TCW):
                    tw = c.TCW
                    wsl = slice(icw * tw, (icw + 1) * tw)
                    # h cast f16->f8 in-flight on the gpsimd DMA queue
                    # (runs in the post-RS quiet window)
                    h8 = p3.tile([128, c.DT, tw], F8, tag="h8", name="h8")
                    for hc in range(4):
                        nc.gpsimd.dma_start(
                            h8[:, hc * 4:(hc + 1) * 4],
                            sc[f"h_{s}"][:, hc * 4:(hc + 1) * 4, wsl])

                    res_all = p3.tile([128, c.DT, tw], F16, tag="resa",
                                      name="res_all")
                    nc.sync.dma_start(res_all[:],
                                      ins[f"res_{s}"][:, :, wsl])

                    r_all = p3.tile([128, c.DT, tw], F32, tag="r",
                                    name="r_all")
                    nacc = p3.tile([128, tw], F16, tag="nacc",
                                   name="nacc")
                    zg = p3.tile([128, c.FT, tw], F8, tag="zg", name="zg")

                    for fb in range(c.FT // 2):
                        if s == "x" and icw == 0 and fb < NPF:
                            w1, w3 = pf[s][f"w1{fb}"], pf[s][f"w3{fb}"]
                        else:
                            w1 = p3w.tile([128, 2, c.DT, 128], F8,
                                          tag="w1", bufs=6, name="w1")
                            nc.sync.dma_start(
                                w1[:], ins[f"w1P_{s}"][:, 2 * fb:2 * fb + 2])
                            w3 = p3w.tile([128, 2, c.DT, 128], F8,
                                          tag="w3", bufs=6, name="w3")
                            nc.sync.dma_start(
                                w3[:], ins[f"w3P_{s}"][:, 2 * fb:2 * fb + 2])
                        for fi in range(2):
                            ftl = fb * 2 + fi
                            z1 = mm([128, tw], "z1")
                            z3 = mm([128, tw], "z3")
                            for op in range(c.DT // 2):
                                nc.tensor.matmul(
                                    z1[:], w1[:, fi, 2 * op:2 * op + 2, :],
                                    h8[:, 2 * op:2 * op + 2, :],
                                    start=(op == 0),
                                    stop=(op == c.DT // 2 - 1),
                                    perf_mode=DR)
                            for op in range(c.DT // 2):
                                nc.tensor.matmul(
                                    z3[:], w3[:, fi, 2 * op:2 * op + 2, :],
                                    h8[:, 2 * op:2 * op + 2, :],
                                    start=(op == 0),
                                    stop=(op == c.DT // 2 - 1),
                                    perf_mode=DR)
                            sg = p3s.tile([128, tw], F16, tag="sg",
                                          name="sg")
                            nc.scalar.activation(sg[:], z1[:], AF.Sigmoid,
                                                 scale=1.0 / WS1)
                            sl = p3s.tile([128, tw], F16, tag="sl",
                                          name="sl")
                            nc.vector.tensor_mul(sl[:], z1[:], sg[:])
                            # zg8 = z3*sl/(WS1*WS1*WS2) = zg_true/WS2
                            nc.vector.scalar_tensor_tensor(
                                zg[:, ftl], z3[:],
                                1.0 / (WS1 * WS1 * WS2), sl[:],
                                MUL, MUL)

                    for db in range(c.DT // 2):
                        w2 = p3w.tile([128, 2, c.FT, 128], F8, tag="w2",
                                      name="w2")
                        nc.gpsimd.dma_start(
                            w2[:], ins[f"w2P_{s}"][:, 2 * db:2 * db + 2])
                        for di in range(2):
                            dt = db * 2 + di
                            fp = mm([128, tw], "fp")
                            for fpair in range(c.FT // 2):
                                nc.tensor.matmul(
                                    fp[:],
                                    w2[:, di, 2 * fpair:2 * fpair + 2, :],
                                    zg[:, 2 * fpair:2 * fpair + 2, :],
                                    start=(fpair == 0),
                                    stop=(fpair == c.FT // 2 - 1),
                                    perf_mode=DR)
                            nc.vector.tensor_add(r_all[:, dt], fp[:],
                                                 res_all[:, dt])
                            r2 = p3s.tile([128, tw], F16, tag="r2",
                                          name="r2")
                            nc.vector.tensor_mul(r2[:], r_all[:, dt],
                                                 r_all[:, dt])
                            if dt == 0:
                                nc.vector.tensor_copy(nacc[:], r2[:])
                            else:
                                nc.vector.tensor_add(nacc[:], nacc[:],
                                                     r2[:])
                    bcn = col_rsqrt(nacc, tw, p3s, f"fn{s}", one_over_d)
                    for dt in range(c.DT):
                        nc.vector.tensor_mul(r_all[:, dt], r_all[:, dt],
                                             bcn[:])
                        ofn = p3s.tile([128, tw], F32, tag="ofn",
                                       name="ofn")
                        nc.scalar.activation(ofn[:], r_all[:, dt], AF.Copy,
                                             scale=fnorm[:, dt:dt + 1])
                        nc.sync.dma_start(
                            outs[s][dt * 128:(dt + 1) * 128, wsl], ofn[:])


# ======================= host-side wrapper =========================

_CACHE = {}


def _prep_inputs(cfg, x, y, attn_norm_w,
                 wq_x, wk_x, wv_x, wo_x, wq_y, wk_y, wv_y, wo_y,
                 w1_x, w2_x, w3_x, ffn_norm_x,
                 w1_y, w2_y, w3_y, ffn_norm_y):
    import ml_dtypes
    c = cfg
    f16 = np.float16
    f8 = ml_dtypes.float8_e4m3
    nw = np.asarray(attn_norm_w, np.float32)
    qscale = nw / np.sqrt(c.HD)

    per_core = [dict() for _ in range(NCORES)]
    shared = {}
    for s, (xv, wq, wk, wv, wo, w1, w2, w3, fn) in {
        "x": (x, wq_x, wk_x, wv_x, wo_x, w1_x, w2_x, w3_x, ffn_norm_x),
        "y": (y, wq_y, wk_y, wv_y, wo_y, w1_y, w2_y, w3_y, ffn_norm_y),
    }.items():
        xt = np.asarray(xv, np.float32).reshape(c.T, c.D).T  # [D, T]
        xt16 = np.ascontiguousarray(xt).astype(f16)
        # chunk-blocked p-major activations: [128, T/TCH, DT, TCH] fp8
        shared[f"{s}P"] = np.ascontiguousarray(
            xt.reshape(c.DT, 128, c.T // c.TCH, c.TCH)
            .transpose(1, 2, 0, 3)).astype(f8)
        wqT = (np.asarray(wq, np.float32) * qscale[None, :]).T  # [D, D]
        wkT = (np.asarray(wk, np.float32) * nw[None, :]).T
        wvT = (np.asarray(wv, np.float32) * nw[None, :]).T
        woT = np.asarray(wo, np.float32).T                     # [Din, Dout]
        # fp8 DoubleRow FFN weights, pair-sliceable on the k dim:
        #   w1/w3: [128, FT, DT, 128] = w.T[D,FF] blocked (d-tile, f-tile)
        #   w2:    [128, DT, FT, 128] = w.T[FF,D] blocked (f-tile, d-tile)
        DT, FT = c.DT, c.FT
        w1T = np.asarray(w1, np.float32).T * WS1
        w3T = np.asarray(w3, np.float32).T * WS1
        w2T = np.asarray(w2, np.float32).T * WS2
        shared[f"w1P_{s}"] = np.ascontiguousarray(
            w1T.reshape(DT, 128, FT, 128).transpose(1, 2, 0, 3)).astype(f8)
        shared[f"w3P_{s}"] = np.ascontiguousarray(
            w3T.reshape(DT, 128, FT, 128).transpose(1, 2, 0, 3)).astype(f8)
        shared[f"w2P_{s}"] = np.ascontiguousarray(
            w2T.reshape(FT, 128, DT, 128).transpose(1, 2, 0, 3)).astype(f8)
        shared[f"fnorm_{s}"] = np.ascontiguousarray(
            np.asarray(fn, np.float32).reshape(c.DT, 128).T)
        for r in range(NCORES):
            js = slice(r * c.NQ, (r + 1) * c.NQ)
            ts = slice(r * c.TC, (r + 1) * c.TC)
            def ptile_w(m):  # [D, NQ] -> [128, DT, NQ] fp8, scaled
                return np.ascontiguousarray(
                    (m * WSQ).reshape(c.DT, 128, c.NQ)
                    .transpose(1, 0, 2)).astype(f8)
            per_core[r][f"wqP_{s}"] = ptile_w(wqT[:, js])
            per_core[r][f"wkP_{s}"] = ptile_w(wkT[:, js])
            per_core[r][f"wvP_{s}"] = ptile_w(wvT[:, js])
            per_core[r][f"woP_{s}"] = np.ascontiguousarray(
                woT[js, :].astype(f16).reshape(c.NQT, 128, c.D)
                .transpose(1, 0, 2))
            per_core[r][f"res_{s}"] = np.ascontiguousarray(
                xt16[:, ts].reshape(c.DT, 128, c.TC).transpose(1, 0, 2))
    in_maps = []
    for r in range(NCORES):
        m = dict(shared)
        m.update(per_core[r])
        in_maps.append(m)
    return in_maps


def run(cfg, inputs, **kw):
    from concourse import bass_utils

    key = (cfg.B, cfg.S, cfg.D, cfg.H, cfg.HD, cfg.FF)
    if key not in _CACHE:
        _CACHE[key] = build(cfg)
    nc = _CACHE[key]
    in_maps = _prep_inputs(cfg, **{k: v for k, v in inputs.items()
                                   if k != "start_pos"})
    res = bass_utils.run_bass_kernel_spmd(
        nc, in_maps, core_ids=list(range(NCORES)), **kw)
    outs = []
    for s in ("x", "y"):
        cols = [res.results[r][f"out_{s}"] for r in range(NCORES)]
        full_t = np.concatenate(cols, axis=1)           # [D, T]
        outs.append(np.ascontiguousarray(full_t.T)
                    .reshape(cfg.B, cfg.S, cfg.D).astype(np.float32))
    return tuple(outs), res


def kernel(**inputs):
    (out_x, out_y), _ = run(FULL, inputs)
    return out_x, out_y
